# revision 9
# baseline (speedup 1.0000x reference)
"""Trainium2 Bass kernel for nn_DecorrelationPatch2d.

reference = fold(unfold(x) * R.sum(1)) / fold(unfold(ones)) collapses to
out[n,c,h,w] = x[n,c,h,w] * W[c,h,w] with W[c] = Bh' @ S_c @ Bw'^T
(rank-3 separable window-average of s = R.sum(1)); see _host_tables.
Channels sharded 8-per-core across 8 cores; device does an elementwise
multiply at the HBM roofline. W generated on-device via tiny PE matmuls.

W[c] = Bh' @ S_c @ Bw'^T  (rank-3 separable structure), where
Bh'[h,i] = [i in Vh(h)]/|Vh(h)|, Bw'[w,j] = [j in Vw(w)]/|Vw(w)|,
S_c = R.sum(1).reshape(C,3,3)[c].  Host ships a single [3, 280] f32
tensor (S^T per channel | Bw'^T | Bh'^T) ~ 3.3KB instead of a 512KB
W map; the elementwise multiply reads W straight out of PSUM.
"""

import numpy as np

import concourse.bass as bass
from concourse import mybir
from concourse.bass_utils import run_bass_kernel_spmd

N, C, H, W = 8, 64, 128, 128
KH = KW = 3
NCORES = 8
CS = C // NCORES
FW = CS * W  # 1024
FX = N * FW  # 8192
WCOLS = 3 * CS + 2 * W  # 24 + 256 = 280

_NC_CACHE = {}


def _build_nc(loop: int = 1):
    """Build the kernel module. loop>1 repeats the body in-NEFF (barrier
    separated) — used only for benchmarking marginal per-body HW time."""
    key = ("nc", loop)
    if key in _NC_CACHE:
        return _NC_CACHE[key]
    f32 = mybir.dt.float32
    nc = bass.Bass()
    xt = nc.dram_tensor("xt", [H, FX], f32, kind="ExternalInput")
    winp = nc.dram_tensor("winp", [KH, WCOLS], f32, kind="ExternalInput")
    out = nc.dram_tensor("out", [H, FX], f32, kind="ExternalOutput")

    with (
        nc.Block() as block,
        nc.semaphore("w_sem") as w_sem,
        nc.semaphore("in_sem") as in_sem,
        nc.semaphore("mm1_sem") as mm1_sem,
        nc.semaphore("mm2_sem") as mm2_sem,
        nc.semaphore("q_sem") as q_sem,
        nc.semaphore("comp_sem") as comp_sem,
        nc.semaphore("out_sem") as out_sem,
        nc.sbuf_tensor("wib", [KH, WCOLS], f32) as wib,
        nc.sbuf_tensor("qs", [KH, FW], f32) as qs,
        nc.sbuf_tensor("spacer", [1, 1], f32) as spacer,
        nc.sbuf_tensor("xbuf", [H, FX], f32) as xbuf,
        nc.sbuf_tensor("ybuf", [H, FX], f32) as ybuf,
        nc.psum_tensor("QP", [KH, FW], f32) as QP,
        nc.psum_tensor("WP", [H, FW], f32) as WP,
    ):
        BW_OFF = 3 * CS  # Bw'^T columns
        BH_OFF = 3 * CS + W  # Bh'^T columns

        for it in range(loop):
            first = it == 0
            if first:

                @block.sync
                def _(sync: bass.BassEngine):
                    sync.dma_start(out=wib[:, :], in_=winp[:, :]).then_inc(w_sem, 16)
                    for n in range(N):
                        sl = slice(n * FW, (n + 1) * FW)
                        sync.dma_start(out=xbuf[:, sl], in_=xt[:, sl]).then_inc(
                            in_sem, 16
                        )

                @block.tensor
                def _(tensor: bass.BassEngine):
                    tensor.wait_ge(w_sem, 16)
                    for c in range(CS):
                        # Q_c = S_c @ Bw'^T  -> [3, 128]
                        tensor.matmul(
                            QP[:, c * W : (c + 1) * W],
                            wib[:, 3 * c : 3 * (c + 1)],
                            wib[:, BW_OFF : BW_OFF + W],
                            start=True,
                            stop=True,
                        ).then_inc(mm1_sem, 1)
                    tensor.wait_ge(q_sem, 1)
                    for c in range(CS):
                        # W_c = Bh' @ Q_c -> [128, 128]
                        tensor.matmul(
                            WP[:, c * W : (c + 1) * W],
                            wib[:, BH_OFF : BH_OFF + W],
                            qs[:, c * W : (c + 1) * W],
                            start=True,
                            stop=True,
                        ).then_inc(mm2_sem, 1)

                @block.vector
                def _(vector: bass.BassEngine):
                    vector.wait_ge(mm1_sem, CS)
                    vector.tensor_copy(qs[:, :], QP[:, :]).then_inc(q_sem, 1)
                    # spacer absorbs the mm2 wait so each TT carries one wait
                    vector.wait_ge(mm2_sem, CS)
                    vector.tensor_copy(spacer[:, :], qs[0:1, 0:1])
                    for n in range(N):
                        sl = slice(n * FW, (n + 1) * FW)
                        vector.wait_ge(in_sem, 16 * (n + 1))
                        vector.tensor_mul(
                            ybuf[:, sl], xbuf[:, sl], WP[:, :]
                        ).then_inc(comp_sem, 1)

            else:
                # benchmark-only repeat: W already in PSUM; cumulative
                # thresholds handle cross-iteration RAW/WAR with one wait
                # per instruction (spacer copies absorb the WAR waits).
                @block.sync
                def _(sync: bass.BassEngine):
                    for n in range(N):
                        sl = slice(n * FW, (n + 1) * FW)
                        sync.wait_ge(comp_sem, N * (it - 1) + n + 1)
                        sync.dma_start(out=xbuf[:, sl], in_=xt[:, sl]).then_inc(
                            in_sem, 16
                        )

                @block.vector
                def _(vector: bass.BassEngine):
                    for n in range(N):
                        sl = slice(n * FW, (n + 1) * FW)
                        vector.wait_ge(out_sem, 16 * (N * (it - 1) + n + 1))
                        vector.tensor_copy(spacer[:, :], qs[0:1, 0:1])
                        vector.wait_ge(in_sem, 16 * (N * it + n + 1))
                        vector.tensor_mul(
                            ybuf[:, sl], xbuf[:, sl], WP[:, :]
                        ).then_inc(comp_sem, 1)

            @block.scalar
            def _(scalar: bass.BassEngine):
                for n in range(N):
                    sl = slice(n * FW, (n + 1) * FW)
                    scalar.wait_ge(comp_sem, N * it + n + 1)
                    scalar.dma_start(out=out[:, sl], in_=ybuf[:, sl]).then_inc(
                        out_sem, 16
                    )
                if it == loop - 1:
                    scalar.wait_ge(out_sem, 16 * N * loop)
                    # out_sem==16*N*loop proves every wait in the program has
                    # been passed and every DMA has retired; clear sems so the
                    # loaded NEFF can be re-executed (PJRT keeps it loaded
                    # across kernel() calls).
                    sems = (
                        w_sem,
                        in_sem,
                        mm1_sem,
                        mm2_sem,
                        q_sem,
                        comp_sem,
                        out_sem,
                    )
                    nums = sorted(s.num for s in sems)
                    if nums == list(range(nums[0], nums[0] + len(nums))):
                        scalar.sem_clear(range(nums[0], nums[-1] + 1))
                    else:
                        for s in sems:
                            scalar.sem_clear(s)

    _NC_CACHE[key] = nc
    return nc


def _host_tables(R: np.ndarray):
    """Per-core [3, 280] tensors: [S_c^T | Bw'^T | Bh'^T]."""
    s = np.asarray(R, np.float64).sum(axis=1).reshape(C, KH, KW).astype(np.float32)
    idx = np.arange(H)
    lo = np.maximum(0, idx - (H - KH))
    hi = np.minimum(KH - 1, idx)
    B = ((np.arange(KH)[None, :] >= lo[:, None]) & (np.arange(KH)[None, :] <= hi[:, None])).astype(np.float32)
    r = (1.0 / (hi - lo + 1)).astype(np.float32)
    Bp = B * r[:, None]  # [H, 3] = Bh' == Bw' (H == W, KH == KW)
    BpT = np.ascontiguousarray(Bp.T)  # [3, H]
    tables = []
    for k in range(NCORES):
        t = np.zeros((KH, WCOLS), np.float32)
        for c in range(CS):
            # t[j, 3c+i] = s[ch, i, j]
            t[:, 3 * c : 3 * (c + 1)] = s[k * CS + c].T
        t[:, 3 * CS : 3 * CS + W] = BpT
        t[:, 3 * CS + W :] = BpT
        tables.append(t)
    return tables


def kernel(x, R):
    x = np.ascontiguousarray(np.asarray(x, dtype=np.float32))
    R = np.asarray(R, dtype=np.float32)
    tables = _host_tables(R)

    xT = np.ascontiguousarray(x.transpose(2, 0, 1, 3))  # [H, N, C, W]
    in_maps = []
    for k in range(NCORES):
        xs = np.ascontiguousarray(xT[:, :, k * CS : (k + 1) * CS, :]).reshape(H, FX)
        in_maps.append({"xt": xs, "winp": tables[k]})

    nc = _build_nc()
    res = run_bass_kernel_spmd(nc, in_maps, core_ids=list(range(NCORES)))

    out = np.empty_like(x)
    for k in range(NCORES):
        blk = res.results[k]["out"].reshape(H, N, CS, W).transpose(1, 2, 0, 3)
        out[:, k * CS : (k + 1) * CS] = blk
    return out
